# revision 9
# baseline (speedup 1.0000x reference)
"""CrossAttention kernel for 8 Trainium2 NeuronCores.

Problem (hardcoded shapes): B=4, N=1024, C=1024, E=1024, H=16, D=64.
  kv = x @ Wkv + bkv ; k, v = split(kv) ; q = query @ Wq + bq
  keys = [k; q] (2N), values = [v; v]
  out = softmax(q keys^T / sqrt(D)) @ values        -> [B, N, E]

Sharding: 8 cores = 4 batches x 2 head-groups (8 heads each).

Per-core design (ACT-exp is the roofline; everything else hides under it):
  - bf16 inputs/weights (host-cast, host-pretiled to SBUF layouts).
  - attention runs per head-pair; the q-as-keys half of the key range runs
    FIRST so exp starts as soon as q^T is projected (x may still be loading).
  - values are duplicated across the two key halves, so PV contracts over
    probs1+probs2 (one bf16 DVE add per tile) - half the PV matmul work.
  - PV orientation out[q, d]: stationary = summed probs [keys,128q] bf16,
    moving = v-tile [keys, 65] bf16 (65th col = ones -> softmax denominator).
    16 accumulators/pair packed 7/7/2 into 3 PSUM banks (73-elem slots so no
    matmul output crosses a bank).
  - scores psum double-buffered (4 banks), next-pair projections accumulate
    in 1 rotating bank; 4+3+1 = 8 banks exactly.
  - biases folded into the PSUM->SBUF copies on DVE (per-partition scalar for
    k/q, broadcast row for v); normalization = reciprocal of the denominator
    column + per-partition scalar multiply on DVE; output stored [N, EC]
    directly (no host transpose).
"""
import numpy as np

B, N, C, E, H = 4, 1024, 1024, 1024, 16
D = E // H            # 64
HPC = 8               # heads per core
EC = HPC * D          # 512 E-columns per core
NCORES = 8
CT = C // 128         # 8 contraction tiles
ST = N // 128         # 8 seq tiles
PAIRS = HPC // 2      # 4 head pairs
KB = N // 128         # 8 key blocks per key half

_compiled = None


def _build():
    import concourse.bass as bass
    import concourse.bacc as bacc
    import concourse.mybir as mybir
    import concourse.tile as tile
    import contextlib
    from collections import deque

    F32 = mybir.dt.float32
    F32R = mybir.dt.float32r
    BF16 = mybir.dt.bfloat16
    EXP = mybir.ActivationFunctionType.Exp
    ADD = mybir.AluOpType.add
    MULT = mybir.AluOpType.mult

    nc = bacc.Bacc()
    xT_in = nc.declare_dram_parameter("xT", [128, CT, N], BF16, isOutput=False)
    qryT_in = nc.declare_dram_parameter("qryT", [128, CT, N], BF16, isOutput=False)
    wq_in = nc.declare_dram_parameter("wq", [128, PAIRS, CT, 128], BF16, isOutput=False)
    wk_in = nc.declare_dram_parameter("wk", [128, PAIRS, CT, 128], BF16, isOutput=False)
    wv_in = nc.declare_dram_parameter("wv", [128, PAIRS, CT, 128], BF16, isOutput=False)
    bq_in = nc.declare_dram_parameter("bqc", [128, PAIRS], F32, isOutput=False)
    bk_in = nc.declare_dram_parameter("bkc", [128, PAIRS], F32, isOutput=False)
    bv_in = nc.declare_dram_parameter("bvv", [128, EC], BF16, isOutput=False)
    out_o = nc.declare_dram_parameter("out_t", [N, EC], F32, isOutput=True)

    with tile.TileContext(nc) as tc, contextlib.ExitStack() as ctx:
        pers = ctx.enter_context(tc.tile_pool(name="pers", bufs=1))
        ekp = ctx.enter_context(tc.tile_pool(name="ekp", bufs=3))
        esp = ctx.enter_context(tc.tile_pool(name="esp", bufs=3))
        outp = ctx.enter_context(tc.tile_pool(name="outp", bufs=3))
        prj = ctx.enter_context(tc.tile_pool(name="prj", bufs=1, space="PSUM"))
        scp = ctx.enter_context(tc.tile_pool(name="scp", bufs=2, space="PSUM"))
        pvp = ctx.enter_context(tc.tile_pool(name="pvp", bufs=1, space="PSUM"))

        # ---- persistent SBUF ----
        xTs = pers.tile([128, CT, N], BF16, tag="xTs")
        qryTs = pers.tile([128, CT, N], BF16, tag="qryTs")
        wqs = pers.tile([128, PAIRS, CT, 128], BF16, tag="wqs")
        wks = pers.tile([128, PAIRS, CT, 128], BF16, tag="wks")
        wvs = pers.tile([128, PAIRS, CT, 128], BF16, tag="wvs")
        qTs = pers.tile([128, PAIRS, N], BF16, tag="qTs")
        kTs = pers.tile([128, PAIRS, N], BF16, tag="kTs")
        vvs = pers.tile([128, ST, HPC, 66], BF16, tag="vvs")
        bqr = pers.tile([128, PAIRS], F32, tag="bqr")
        bkr = pers.tile([128, PAIRS], F32, tag="bkr")
        bvr = pers.tile([128, EC], BF16, tag="bvr")
        # q-part probs for the current pair (overwritten each pair)
        eqs = pers.tile([128, KB, 2, N], BF16, tag="eqs")

        # ---- loads, priority order (DMA is serial): biases, then the
        # q-projection chain (wq pair0 -> qryT), then k (wk p0 -> xT), v,
        # then remaining pairs' weights.
        nc.sync.dma_start(out=bqr[:], in_=bq_in[:, :])
        nc.sync.dma_start(out=bkr[:], in_=bk_in[:, :])
        nc.sync.dma_start(out=bvr[:], in_=bv_in[:, :])
        nc.sync.dma_start(out=wqs[:, 0], in_=wq_in[:, 0])
        for ct in range(CT):
            nc.sync.dma_start(out=qryTs[:, ct], in_=qryT_in[:, ct])
        nc.sync.dma_start(out=wks[:, 0], in_=wk_in[:, 0])
        for ct in range(CT):
            nc.sync.dma_start(out=xTs[:, ct], in_=xT_in[:, ct])
        nc.sync.dma_start(out=wvs[:, 0], in_=wv_in[:, 0])
        for p in range(1, PAIRS):
            nc.sync.dma_start(out=wqs[:, p], in_=wq_in[:, p])
            nc.sync.dma_start(out=wks[:, p], in_=wk_in[:, p])
            nc.sync.dma_start(out=wvs[:, p], in_=wv_in[:, p])

        wrm = pers.tile([128, 512], BF16, tag="wrm")
        nc.vector.memset(wrm[:], 0.5)
        nc.vector.memset(vvs[:, :, :, 64:65], 1.0)
        # PE p-state warmup: the cost model ramps the PE clock from 0.65GHz to
        # 2.4GHz over ~3us of busy time measured from the first matmul; burn
        # that ramp on dummies while the DMAs land so the real projections run
        # at full clock.
        for _ in range(8):
            pw = prj.tile([128, 512], F32, tag="prj", name="pw")
            nc.tensor.matmul(pw[:], wrm[:, 0:128], wrm[:], start=True, stop=True)

        # ---- projection emitters (yield every couple of matmuls so they can
        # be pumped into the PE stream between attention steps) ----
        def gen_1_proj(p, wsb, src, dstT, brow):
            for half in range(2):
                hsl = slice(half * 512, (half + 1) * 512)
                pt = prj.tile([128, 512], F32, tag="prj")
                for ct in range(CT):
                    nc.tensor.matmul(pt[:], wsb[:, p, ct, :],
                                     src[:, ct, hsl],
                                     start=(ct == 0), stop=(ct == CT - 1))
                    if ct % 2 == 1:
                        yield
                nc.vector.tensor_scalar(out=dstT[:, p, hsl], in0=pt[:],
                                        scalar1=brow[:, p:p + 1],
                                        scalar2=None, op0=ADD)
                yield

        def gen_q_proj(p):
            yield from gen_1_proj(p, wqs, qryTs, qTs, bqr)

        def gen_k_proj(p):
            yield from gen_1_proj(p, wks, xTs, kTs, bkr)

        def gen_v_proj(p):
            for g in range(2):
                pt = prj.tile([128, 4, 128], F32, tag="prj")
                for ct in range(CT):
                    for si in range(4):
                        st = g * 4 + si
                        # start=True zeroes the WHOLE psum bank: only the
                        # first matmul of the bank starts; siblings accumulate
                        # onto the zeroed bank.
                        nc.tensor.matmul(pt[:, si, :],
                                         xTs[:, ct, st * 128:(st + 1) * 128],
                                         wvs[:, p, ct, :],
                                         start=(ct == 0 and si == 0),
                                         stop=(ct == CT - 1),
                                         skip_group_check=True)
                    yield
                for si in range(4):
                    st = g * 4 + si
                    nc.vector.tensor_add(
                        out=vvs[:, st, 2 * p:2 * p + 2, 0:64],
                        in0=pt[:, si, :].rearrange("q (h d) -> q h d", h=2),
                        in1=bvr[:, p * 128:(p + 1) * 128].rearrange(
                            "q (h d) -> q h d", h=2))
                yield

        bg = deque()

        def pump(n):
            for _ in range(n):
                while bg:
                    try:
                        next(bg[0])
                        break
                    except StopIteration:
                        bg.popleft()
                else:
                    return

        # PV accumulators: 16 accs (hi*8+qc) packed 7/7/2 into 3 banks.
        def acc_of(tiles, j):
            if j < 7:
                return tiles[0], j
            if j < 14:
                return tiles[1], j - 7
            return tiles[2], j - 14

        HI = ((0, slice(0, 64)), (1, slice(64, 128)))

        def emit_scores_exp(p, src, kb, dst):
            ksl = slice(kb * 128, (kb + 1) * 128)
            out = []
            for hi, rows in HI:
                sct = scp.tile([128, N], F32, tag="sc")
                nc.tensor.matmul(sct[:, 0:512], src[rows, p, ksl],
                                 qTs[rows, p, 0:512])
                nc.tensor.matmul(sct[:, 512:1024], src[rows, p, ksl],
                                 qTs[rows, p, 512:1024])
                e = dst(hi)
                nc.scalar.activation(out=e, in_=sct[:], func=EXP, scale=0.125)
                out.append(e)
            return out

        def emit_pv(p, kb, es2, start, stop):
            for hi in range(2):
                for qc in range(8):
                    j = hi * 8 + qc
                    t, jj = acc_of(pv_tiles, j)
                    # start=True zeroes the whole bank; only the first acc of
                    # each of the 3 banks starts (j = 0 / 7 / 14).
                    nc.tensor.matmul(t[:, jj, 0:65],
                                     es2[hi][:, qc * 128:(qc + 1) * 128],
                                     vvs[:, kb, 2 * p + hi, 0:65],
                                     start=(start and j in (0, 7, 14)),
                                     stop=stop, skip_group_check=True)

        # ---- head: only pair 0's q projection runs eagerly (its DMA chain
        # loads first); k/v of pair 0 and everything for later pairs pump
        # through the PE's slack during attention steps.
        for _ in gen_q_proj(0):
            pass
        bg.append(gen_k_proj(0))
        bg.append(gen_v_proj(0))
        for np_ in range(1, PAIRS):
            bg.append(gen_q_proj(np_))
            bg.append(gen_k_proj(np_))
            bg.append(gen_v_proj(np_))

        for p in range(PAIRS):
            # q-as-keys half first: needs only qTs
            for kb in range(KB):
                emit_scores_exp(p, qTs, kb,
                                lambda hi, kb=kb: eqs[:, kb, hi, :])
                pump(0 if kb >= KB - 1 else 4)
            # k half, PV pipelined one step behind
            pv_tiles = (pvp.tile([128, 7, 73], F32, tag="pvA", name="pvA"),
                        pvp.tile([128, 7, 73], F32, tag="pvB", name="pvB"),
                        pvp.tile([128, 2, 73], F32, tag="pvC", name="pvC"))
            pend = None
            for kb in range(KB):
                cur = []

                def dst(hi):
                    e = ekp.tile([128, N], BF16, tag="ek")
                    return e
                es_hi = emit_scores_exp(p, kTs, kb, dst)
                for hi in range(2):
                    est = esp.tile([128, N], BF16, tag="es")
                    nc.vector.tensor_add(out=est[:], in0=es_hi[hi],
                                         in1=eqs[:, kb, hi, :])
                    cur.append(est)
                if pend is not None:
                    emit_pv(p, kb - 1, pend, start=(kb == 1), stop=False)
                pend = cur
                pump(0 if (kb == 0 or kb >= KB - 1) else 4)
            emit_pv(p, KB - 1, pend, start=False, stop=True)

            # normalize + store [N, EC] directly
            rcps = (outp.tile([128, 7, 1], F32, tag="rA", name="rA"),
                    outp.tile([128, 7, 1], F32, tag="rB", name="rB"),
                    outp.tile([128, 2, 1], F32, tag="rC", name="rC"))
            for t, r in zip(pv_tiles, rcps):
                nc.vector.reciprocal(out=r[:], in_=t[:, :, 64:65])
            ost = outp.tile([128, ST, 128], F32, tag="osb")
            for qc in range(8):
                for hi in range(2):
                    t, jj = acc_of(pv_tiles, hi * 8 + qc)
                    r = rcps[0] if hi * 8 + qc < 7 else (
                        rcps[1] if hi * 8 + qc < 14 else rcps[2])
                    nc.vector.tensor_scalar(
                        out=ost[:, qc, hi * 64:(hi + 1) * 64],
                        in0=t[:, jj, 0:64], scalar1=r[:, jj, :],
                        scalar2=None, op0=MULT)
            nc.sync.dma_start(
                out=out_o[:, p * 128:(p + 1) * 128].rearrange(
                    "(qc qi) c -> qi qc c", qi=128),
                in_=ost[:])

    nc.finalize()
    return nc


def _get_compiled():
    global _compiled
    if _compiled is None:
        _compiled = _build()
    return _compiled


def kernel(x, query, Wkv, bkv, Wq, bq):
    import ml_dtypes
    from concourse.bass_utils import run_bass_kernel_spmd

    bf16 = ml_dtypes.bfloat16
    x = np.asarray(x, dtype=np.float32)
    query = np.asarray(query, dtype=np.float32)
    Wkv = np.asarray(Wkv, dtype=np.float32)
    bkv = np.asarray(bkv, dtype=np.float32)
    Wq = np.asarray(Wq, dtype=np.float32)
    bq = np.asarray(bq, dtype=np.float32)

    def tile_T(a):  # [N, C] -> [128, CT, N] (a.T tiled over contraction)
        return np.ascontiguousarray(
            a.T.reshape(CT, 128, N).transpose(1, 0, 2)).astype(bf16)

    def tile_w(w):  # [C, EC] -> [128, PAIRS, CT, 128]
        return np.ascontiguousarray(
            w.reshape(CT, 128, PAIRS, 128).transpose(1, 2, 0, 3)).astype(bf16)

    in_maps = []
    for core in range(NCORES):
        b, hg = core // 2, core % 2
        ecs = slice(hg * EC, (hg + 1) * EC)
        bv = bkv[E + hg * EC:E + (hg + 1) * EC]
        in_maps.append({
            "xT": tile_T(x[b]),
            "qryT": tile_T(query[b]),
            "wq": tile_w(Wq[:, ecs]),
            "wk": tile_w(Wkv[:, hg * EC:(hg + 1) * EC]),
            "wv": tile_w(Wkv[:, E + hg * EC:E + (hg + 1) * EC]),
            "bqc": np.ascontiguousarray(bq[ecs].reshape(PAIRS, 128).T),
            "bkc": np.ascontiguousarray(
                bkv[hg * EC:(hg + 1) * EC].reshape(PAIRS, 128).T),
            "bvv": np.ascontiguousarray(
                np.tile(bv[None, :], (128, 1)).astype(bf16)),
        })

    nc = _get_compiled()
    res = None
    last_err = None
    for attempt in range(3):
        try:
            res = run_bass_kernel_spmd(nc, in_maps, list(range(NCORES)))
            break
        except Exception as ex:  # transient NRT_EXEC_UNIT_UNRECOVERABLE etc.
            last_err = ex
    if res is None:
        raise last_err

    out = np.empty((B, N, E), np.float32)
    for core in range(NCORES):
        b, hg = core // 2, core % 2
        out[b, :, hg * EC:(hg + 1) * EC] = res.results[core]["out_t"]
    return out


# revision 12
# speedup vs baseline: 1.0562x; 1.0562x over previous
"""CrossAttention kernel for 8 Trainium2 NeuronCores.

Problem (hardcoded shapes): B=4, N=1024, C=1024, E=1024, H=16, D=64.
  kv = x @ Wkv + bkv ; k, v = split(kv) ; q = query @ Wq + bq
  keys = [k; q] (2N), values = [v; v]
  out = softmax(q keys^T / sqrt(D)) @ values        -> [B, N, E]

Sharding: 8 cores = 4 batches x 2 head-groups (8 heads each).

Per-core design (ACT-exp is the roofline; everything else hides under it):
  - bf16 inputs/weights (host-cast, host-pretiled to SBUF layouts).
  - attention runs per head-pair; the q-as-keys half of the key range runs
    FIRST so exp starts as soon as q^T is projected (x may still be loading).
  - values are duplicated across the two key halves, so PV contracts over
    probs1+probs2 (one bf16 DVE add per tile) - half the PV matmul work.
  - PV orientation out[q, d]: stationary = summed probs [keys,128q] bf16,
    moving = v-tile [keys, 65] bf16 (65th col = ones -> softmax denominator).
    16 accumulators/pair packed 7/7/2 into 3 PSUM banks (73-elem slots so no
    matmul output crosses a bank).
  - scores psum double-buffered (4 banks), next-pair projections accumulate
    in 1 rotating bank; 4+3+1 = 8 banks exactly.
  - biases folded into the PSUM->SBUF copies on DVE (per-partition scalar for
    k/q, broadcast row for v); normalization = reciprocal of the denominator
    column + per-partition scalar multiply on DVE; output stored [N, EC]
    directly (no host transpose).
"""
import numpy as np

B, N, C, E, H = 4, 1024, 1024, 1024, 16
D = E // H            # 64
HPC = 8               # heads per core
EC = HPC * D          # 512 E-columns per core
NCORES = 8
CT = C // 128         # 8 contraction tiles
ST = N // 128         # 8 seq tiles
PAIRS = HPC // 2      # 4 head pairs
KB = N // 128         # 8 key blocks per key half

_compiled = None


def _build():
    import concourse.bass as bass
    import concourse.bacc as bacc
    import concourse.mybir as mybir
    import concourse.tile as tile
    import contextlib
    from collections import deque

    F32 = mybir.dt.float32
    F32R = mybir.dt.float32r
    BF16 = mybir.dt.bfloat16
    EXP = mybir.ActivationFunctionType.Exp
    ADD = mybir.AluOpType.add
    MULT = mybir.AluOpType.mult

    nc = bacc.Bacc()
    xT_in = nc.declare_dram_parameter("xT", [128, CT, N], BF16, isOutput=False)
    qryT_in = nc.declare_dram_parameter("qryT", [128, CT, N], BF16, isOutput=False)
    wq_in = nc.declare_dram_parameter("wq", [128, PAIRS, CT, 128], BF16, isOutput=False)
    wk_in = nc.declare_dram_parameter("wk", [128, PAIRS, CT, 128], BF16, isOutput=False)
    wv_in = nc.declare_dram_parameter("wv", [128, PAIRS, CT, 128], BF16, isOutput=False)
    bq_in = nc.declare_dram_parameter("bqc", [128, PAIRS], F32, isOutput=False)
    bk_in = nc.declare_dram_parameter("bkc", [128, PAIRS], F32, isOutput=False)
    bv_in = nc.declare_dram_parameter("bvv", [128, EC], BF16, isOutput=False)
    out_o = nc.declare_dram_parameter("out_t", [N, EC], F32, isOutput=True)

    with tile.TileContext(nc) as tc, contextlib.ExitStack() as ctx:
        pers = ctx.enter_context(tc.tile_pool(name="pers", bufs=1))
        ekp = ctx.enter_context(tc.tile_pool(name="ekp", bufs=3))
        esp = ctx.enter_context(tc.tile_pool(name="esp", bufs=3))
        outp = ctx.enter_context(tc.tile_pool(name="outp", bufs=3))
        prj = ctx.enter_context(tc.tile_pool(name="prj", bufs=1, space="PSUM"))
        scp = ctx.enter_context(tc.tile_pool(name="scp", bufs=2, space="PSUM"))
        pvp = ctx.enter_context(tc.tile_pool(name="pvp", bufs=1, space="PSUM"))

        # ---- persistent SBUF ----
        xTs = pers.tile([128, CT, N], BF16, tag="xTs")
        qryTs = pers.tile([128, CT, N], BF16, tag="qryTs")
        wqs = pers.tile([128, PAIRS, CT, 128], BF16, tag="wqs")
        wks = pers.tile([128, PAIRS, CT, 128], BF16, tag="wks")
        wvs = pers.tile([128, PAIRS, CT, 128], BF16, tag="wvs")
        qTs = pers.tile([128, PAIRS, N], BF16, tag="qTs")
        kTs = pers.tile([128, PAIRS, N], BF16, tag="kTs")
        vvs = pers.tile([128, ST, HPC, 66], BF16, tag="vvs")
        bqr = pers.tile([128, PAIRS], F32, tag="bqr")
        bkr = pers.tile([128, PAIRS], F32, tag="bkr")
        bvr = pers.tile([128, EC], BF16, tag="bvr")
        # q-part probs for the current pair (overwritten each pair)
        eqs = pers.tile([128, KB, 2, N], BF16, tag="eqs")

        # ---- loads, priority order (DMA is serial): biases, then the
        # q-projection chain (wq pair0 -> qryT), then k (wk p0 -> xT), v,
        # then remaining pairs' weights.
        nc.sync.dma_start(out=wqs[:, 0], in_=wq_in[:, 0])
        for c2 in range(CT // 2):
            nc.sync.dma_start(out=qryTs[:, 2 * c2:2 * c2 + 2],
                              in_=qryT_in[:, 2 * c2:2 * c2 + 2])
        nc.sync.dma_start(out=bqr[:], in_=bq_in[:, :])
        nc.sync.dma_start(out=bkr[:], in_=bk_in[:, :])
        nc.sync.dma_start(out=wks[:, 0], in_=wk_in[:, 0])
        for c2 in range(CT // 2):
            nc.sync.dma_start(out=xTs[:, 2 * c2:2 * c2 + 2],
                              in_=xT_in[:, 2 * c2:2 * c2 + 2])
        nc.sync.dma_start(out=bvr[:], in_=bv_in[:, :])
        nc.sync.dma_start(out=wvs[:, 0], in_=wv_in[:, 0])
        for p in range(1, PAIRS):
            nc.sync.dma_start(out=wqs[:, p], in_=wq_in[:, p])
            nc.sync.dma_start(out=wks[:, p], in_=wk_in[:, p])
            nc.sync.dma_start(out=wvs[:, p], in_=wv_in[:, p])

        wrm = pers.tile([128, 512], BF16, tag="wrm")
        nc.vector.memset(wrm[:], 0.5)
        nc.vector.memset(vvs[:, :, :, 64:65], 1.0)
        # PE p-state warmup: the cost model ramps the PE clock from 0.65GHz to
        # 2.4GHz over ~3us of busy time measured from the first matmul; burn
        # that ramp on dummies while the DMAs land so the real projections run
        # at full clock.
        for _ in range(8):
            pw = prj.tile([128, 512], F32, tag="prj", name="pw")
            nc.tensor.matmul(pw[:], wrm[:, 0:128], wrm[:], start=True, stop=True)

        # ---- projection emitters (yield every couple of matmuls so they can
        # be pumped into the PE stream between attention steps) ----
        def gen_1_proj(p, wsb, src, dstT, brow):
            for half in range(2):
                hsl = slice(half * 512, (half + 1) * 512)
                pt = prj.tile([128, 512], F32, tag="prj")
                for ct in range(CT):
                    nc.tensor.matmul(pt[:], wsb[:, p, ct, :],
                                     src[:, ct, hsl],
                                     start=(ct == 0), stop=(ct == CT - 1))
                    yield 214
                nc.vector.tensor_scalar(out=dstT[:, p, hsl], in0=pt[:],
                                        scalar1=brow[:, p:p + 1],
                                        scalar2=None, op0=ADD)
                yield 0

        def gen_q_proj(p):
            yield from gen_1_proj(p, wqs, qryTs, qTs, bqr)

        def gen_k_proj(p):
            yield from gen_1_proj(p, wks, xTs, kTs, bkr)

        def gen_v_proj(p):
            for g in range(2):
                pt = prj.tile([128, 4, 128], F32, tag="prj")
                for ct in range(CT):
                    for si in range(4):
                        st = g * 4 + si
                        # start=True zeroes the WHOLE psum bank: only the
                        # first matmul of the bank starts; siblings accumulate
                        # onto the zeroed bank.
                        nc.tensor.matmul(pt[:, si, :],
                                         xTs[:, ct, st * 128:(st + 1) * 128],
                                         wvs[:, p, ct, :],
                                         start=(ct == 0 and si == 0),
                                         stop=(ct == CT - 1),
                                         skip_group_check=True)
                    yield 214
                for si in range(4):
                    st = g * 4 + si
                    nc.vector.tensor_add(
                        out=vvs[:, st, 2 * p:2 * p + 2, 0:64],
                        in0=pt[:, si, :].rearrange("q (h d) -> q h d", h=2),
                        in1=bvr[:, p * 128:(p + 1) * 128].rearrange(
                            "q (h d) -> q h d", h=2))
                yield 0

        bg = deque()

        def pump(budget_ns):
            # pull background PE work until ~budget_ns of matmul time emitted
            while budget_ns > 0 and bg:
                try:
                    budget_ns -= next(bg[0])
                except StopIteration:
                    bg.popleft()

        # PV accumulators: 16 accs (hi*8+qc) packed 7/7/2 into 3 banks.
        def acc_of(tiles, j):
            if j < 7:
                return tiles[0], j
            if j < 14:
                return tiles[1], j - 7
            return tiles[2], j - 14

        HI = ((0, slice(0, 64)), (1, slice(64, 128)))

        def emit_scores_exp(p, src, kb, dst):
            ksl = slice(kb * 128, (kb + 1) * 128)
            out = []
            for hi, rows in HI:
                sct = scp.tile([128, N], F32, tag="sc")
                nc.tensor.matmul(sct[:, 0:512], src[rows, p, ksl],
                                 qTs[rows, p, 0:512])
                nc.tensor.matmul(sct[:, 512:1024], src[rows, p, ksl],
                                 qTs[rows, p, 512:1024])
                e = dst(hi)
                nc.scalar.activation(out=e, in_=sct[:], func=EXP, scale=0.125)
                out.append(e)
            return out

        def emit_pv(p, kb, es2, start, stop):
            for hi in range(2):
                for qc in range(8):
                    j = hi * 8 + qc
                    t, jj = acc_of(pv_tiles, j)
                    # start=True zeroes the whole bank; only the first acc of
                    # each of the 3 banks starts (j = 0 / 7 / 14).
                    nc.tensor.matmul(t[:, jj, 0:65],
                                     es2[hi][:, qc * 128:(qc + 1) * 128],
                                     vvs[:, kb, 2 * p + hi, 0:65],
                                     start=(start and j in (0, 7, 14)),
                                     stop=stop, skip_group_check=True)

        # ---- head: only pair 0's q projection runs eagerly (its DMA chain
        # loads first); k/v of pair 0 and everything for later pairs pump
        # through the PE's slack during attention steps.
        for _ in gen_q_proj(0):
            pass
        bg.append(gen_k_proj(0))
        bg.append(gen_v_proj(0))
        for np_ in range(1, PAIRS):
            bg.append(gen_q_proj(np_))
            bg.append(gen_k_proj(np_))
            bg.append(gen_v_proj(np_))

        for p in range(PAIRS):
            # q-as-keys half first: needs only qTs
            for kb in range(KB):
                emit_scores_exp(p, qTs, kb,
                                lambda hi, kb=kb: eqs[:, kb, hi, :])
                if kb < KB - 1:
                    pump(1000)
            # k half, PV pipelined one step behind
            pv_tiles = (pvp.tile([128, 7, 73], F32, tag="pvA", name="pvA"),
                        pvp.tile([128, 7, 73], F32, tag="pvB", name="pvB"),
                        pvp.tile([128, 2, 73], F32, tag="pvC", name="pvC"))
            pend = None
            for kb in range(KB):
                cur = []

                def dst(hi):
                    e = ekp.tile([128, N], BF16, tag="ek")
                    return e
                es_hi = emit_scores_exp(p, kTs, kb, dst)
                for hi in range(2):
                    est = esp.tile([128, N], BF16, tag="es")
                    nc.vector.tensor_add(out=est[:], in0=es_hi[hi],
                                         in1=eqs[:, kb, hi, :])
                    cur.append(est)
                if pend is not None:
                    emit_pv(p, kb - 1, pend, start=(kb == 1), stop=False)
                pend = cur
                if 0 < kb < KB - 1:
                    pump(550)
            emit_pv(p, KB - 1, pend, start=False, stop=True)

            # normalize + store [N, EC]; bank A (hi0 accs) finishes first,
            # so its reciprocal+muls go first and the store ships in halves.
            rcps = (outp.tile([128, 7, 1], F32, tag="rA", name="rA"),
                    outp.tile([128, 7, 1], F32, tag="rB", name="rB"),
                    outp.tile([128, 2, 1], F32, tag="rC", name="rC"))
            ost = outp.tile([128, ST, 128], F32, tag="osb")

            def norm_one(j):
                t, jj = acc_of(pv_tiles, j)
                r = rcps[0] if j < 7 else (rcps[1] if j < 14 else rcps[2])
                nc.vector.tensor_scalar(
                    out=ost[:, j % 8, (j // 8) * 64:(j // 8) * 64 + 64],
                    in0=t[:, jj, 0:64], scalar1=r[:, jj, :],
                    scalar2=None, op0=MULT)

            nc.vector.reciprocal(out=rcps[0][:], in_=pv_tiles[0][:, :, 64:65])
            for j in (0, 1, 2, 3):
                norm_one(j)
            nc.vector.reciprocal(out=rcps[1][:], in_=pv_tiles[1][:, :, 64:65])
            for j in (8, 9, 10, 11):
                norm_one(j)
            nc.sync.dma_start(
                out=out_o[0:512, p * 128:(p + 1) * 128].rearrange(
                    "(qc qi) c -> qi qc c", qi=128),
                in_=ost[:, 0:4, :])
            nc.vector.reciprocal(out=rcps[2][:], in_=pv_tiles[2][:, :, 64:65])
            for j in (4, 5, 6, 7, 12, 13, 14, 15):
                norm_one(j)
            nc.sync.dma_start(
                out=out_o[512:1024, p * 128:(p + 1) * 128].rearrange(
                    "(qc qi) c -> qi qc c", qi=128),
                in_=ost[:, 4:8, :])

    nc.finalize()
    return nc


def _get_compiled():
    global _compiled
    if _compiled is None:
        _compiled = _build()
    return _compiled


def kernel(x, query, Wkv, bkv, Wq, bq):
    import ml_dtypes
    from concourse.bass_utils import run_bass_kernel_spmd

    bf16 = ml_dtypes.bfloat16
    x = np.asarray(x, dtype=np.float32)
    query = np.asarray(query, dtype=np.float32)
    Wkv = np.asarray(Wkv, dtype=np.float32)
    bkv = np.asarray(bkv, dtype=np.float32)
    Wq = np.asarray(Wq, dtype=np.float32)
    bq = np.asarray(bq, dtype=np.float32)

    def tile_T(a):  # [N, C] -> [128, CT, N] (a.T tiled over contraction)
        return np.ascontiguousarray(
            a.T.reshape(CT, 128, N).transpose(1, 0, 2)).astype(bf16)

    def tile_w(w):  # [C, EC] -> [128, PAIRS, CT, 128]
        return np.ascontiguousarray(
            w.reshape(CT, 128, PAIRS, 128).transpose(1, 2, 0, 3)).astype(bf16)

    in_maps = []
    for core in range(NCORES):
        b, hg = core // 2, core % 2
        ecs = slice(hg * EC, (hg + 1) * EC)
        bv = bkv[E + hg * EC:E + (hg + 1) * EC]
        in_maps.append({
            "xT": tile_T(x[b]),
            "qryT": tile_T(query[b]),
            "wq": tile_w(Wq[:, ecs]),
            "wk": tile_w(Wkv[:, hg * EC:(hg + 1) * EC]),
            "wv": tile_w(Wkv[:, E + hg * EC:E + (hg + 1) * EC]),
            "bqc": np.ascontiguousarray(bq[ecs].reshape(PAIRS, 128).T),
            "bkc": np.ascontiguousarray(
                bkv[hg * EC:(hg + 1) * EC].reshape(PAIRS, 128).T),
            "bvv": np.ascontiguousarray(
                np.tile(bv[None, :], (128, 1)).astype(bf16)),
        })

    nc = _get_compiled()
    res = None
    last_err = None
    for attempt in range(3):
        try:
            res = run_bass_kernel_spmd(nc, in_maps, list(range(NCORES)))
            break
        except Exception as ex:  # transient NRT_EXEC_UNIT_UNRECOVERABLE etc.
            last_err = ex
    if res is None:
        raise last_err

    out = np.empty((B, N, E), np.float32)
    for core in range(NCORES):
        b, hg = core // 2, core % 2
        out[b, :, hg * EC:(hg + 1) * EC] = res.results[core]["out_t"]
    return out
